# revision 1
# baseline (speedup 1.0000x reference)
"""Grouped Query Attention on 8 TRN2 NeuronCores.

Sharding: batch x s_q-quarter (core c -> batch c//4, query rows
[512*(c%4), 512*(c%4+1))). Each core computes the Q projection for its
512 query rows, the full KV projection for its batch (duplicated across
the 4 cores of that batch -- cheaper than collectives), attention for
all 16 heads over its query rows, and the output projection for a
disjoint [512, 2048] slice of the output. Unsharding is concatenation;
no collectives.

Layouts: all matmuls contract over the SBUF partition dim. Host
pre-transposes x to x^T (and rotates the core's own s_q quarter to the
front so one SPMD program serves all cores -- the s_k order inside
attention is permutation-invariant). Weights are pre-rearranged on host
so every DMA lands [128, ...] with >=2KB contiguous lines. Scores are
computed transposed ([s_k, s_q]) so softmax denominators come from an
M=1 ones-matmul and attn@V needs no transposes; 1/sqrt(128) is folded
into Wq on host. Matmul operands are bitcast to float32r (full PE
speed at free-dim >= 256, ~tf32 precision).
"""

import numpy as np

E = 2048
S = 2048
P = 128
H = 16
G = 4
SQ = 512          # query rows per core
EB = E // P       # 16 e-blocks (contraction tiles)
KV_N = 2 * E // G  # 1024
NCORES = 8

_NC = None
TRACE = False
LAST_RESULT = None


def _build():
    import concourse.bacc as bacc
    import concourse.mybir as mybir
    import concourse.tile as tile
    from concourse.masks import make_identity

    f32 = mybir.dt.float32
    f32r = mybir.dt.float32r
    bf16 = mybir.dt.bfloat16
    EXP = mybir.ActivationFunctionType.Exp

    nc = bacc.Bacc("TRN2", target_bir_lowering=False, debug=False,
                   num_devices=NCORES)

    xt = nc.declare_dram_parameter("xt", [P, EB, S], f32, isOutput=False).ap()
    wq = nc.declare_dram_parameter("wq", [H, P, EB, P], f32, isOutput=False).ap()
    wkv = nc.declare_dram_parameter("wkv", [P, EB, KV_N], f32, isOutput=False).ap()
    wo = nc.declare_dram_parameter("wo", [P, EB, E], f32, isOutput=False).ap()
    bq = nc.declare_dram_parameter("bq", [P, H], f32, isOutput=False).ap()
    bkv = nc.declare_dram_parameter("bkv", [P, 8], f32, isOutput=False).ap()
    bo = nc.declare_dram_parameter("bo", [1, E], f32, isOutput=False).ap()
    out = nc.declare_dram_parameter("out", [SQ, E], f32, isOutput=True).ap()

    def r(ap):
        return ap.bitcast(f32r)

    with tile.TileContext(nc) as tc:
        with tc.tile_pool(name="consts", bufs=1) as cp, \
             tc.tile_pool(name="otp", bufs=1) as otp, \
             tc.tile_pool(name="dram", bufs=1, space="DRAM") as dp:
            ident = cp.tile([P, P], f32, tag="ident")
            make_identity(nc, ident)
            onec = cp.tile([P, 1], bf16, tag="onec")
            nc.vector.memset(onec, 1.0)
            oner = cp.tile([1, P], f32, tag="oner")
            nc.vector.memset(oner, 1.0)
            bq_s = cp.tile([P, H], f32, tag="bqs")
            nc.sync.dma_start(bq_s, bq)
            bkv_s = cp.tile([P, 8], f32, tag="bkvs")
            nc.sync.dma_start(bkv_s, bkv)
            bo_s = cp.tile([1, E], f32, tag="bos")
            nc.sync.dma_start(bo_s, bo)

            OT = otp.tile([P, H, SQ], f32, tag="ot")   # normalized attn out, [hd, head, s_q]
            kvT = dp.tile([8 * P, S], f32, tag="kvt")  # K^T/V^T scratch, rows = kv M-tiles
            qtd = dp.tile([H * P, SQ], f32, tag="qtd") # Q^T scratch, rows = head blocks

            with tc.tile_pool(name="wkvp", bufs=1) as wkvp:
                wkv_s = wkvp.tile([P, EB, KV_N], f32, tag="wkvs")
                nc.sync.dma_start(r(wkv_s), r(wkv))

                # ---- Phase 1a: Q projection for this core's s_q quarter
                # (= chunk 0 of the rotated x^T). QT[do, s_q] accumulated
                # over 16 e-blocks, bias added on PSUM->SBUF, spilled to
                # DRAM scratch (re-streamed per head in phase 2).
                with tc.tile_pool(name="xqp", bufs=1) as xqp, \
                     tc.tile_pool(name="wqp", bufs=2) as wqp, \
                     tc.tile_pool(name="qop", bufs=2) as qop, \
                     tc.tile_pool(name="ps1", bufs=3, space="PSUM") as ps1:
                    xq = xqp.tile([P, EB, SQ], f32, tag="xq")
                    nc.sync.dma_start(r(xq), r(xt[:, :, 0:SQ]))
                    for m in range(H):
                        wqm = wqp.tile([P, EB, P], f32, tag="wqm")
                        nc.sync.dma_start(r(wqm), r(wq[m]))
                        ps = ps1.tile([P, SQ], f32, tag="ps")
                        for b in range(EB):
                            nc.tensor.matmul(ps, r(wqm[:, b]), r(xq[:, b]),
                                             start=(b == 0), stop=(b == EB - 1))
                        qo = qop.tile([P, SQ], f32, tag="qo")
                        nc.vector.tensor_scalar_add(qo, ps, bq_s[:, m:m + 1])
                        nc.sync.dma_start(qtd[m * P:(m + 1) * P, :], qo)

                # ---- Phase 1b: KV projection over the full sequence
                # (4 chunks of 512). M-tiles: [K0 V0 K1 V1 ...] matching
                # host Wkv column order.
                with tc.tile_pool(name="xcp", bufs=2) as xcp, \
                     tc.tile_pool(name="kvo", bufs=3) as kvo, \
                     tc.tile_pool(name="ps1b", bufs=3, space="PSUM") as ps1b:
                    for j in range(4):
                        xc = xcp.tile([P, EB, 512], f32, tag="xc")
                        nc.sync.dma_start(r(xc), r(xt[:, :, 512 * j:512 * (j + 1)]))
                        for m in range(8):
                            ps = ps1b.tile([P, 512], f32, tag="ps")
                            for b in range(EB):
                                nc.tensor.matmul(
                                    ps, r(wkv_s[:, b, m * P:(m + 1) * P]),
                                    r(xc[:, b]),
                                    start=(b == 0), stop=(b == EB - 1))
                            ko = kvo.tile([P, 512], f32, tag="ko")
                            nc.vector.tensor_scalar_add(ko, ps, bkv_s[:, m:m + 1])
                            nc.sync.dma_start(
                                kvT[m * P:(m + 1) * P, 512 * j:512 * (j + 1)], ko)

            # ---- Phase 2: attention, head group by head group.
            # scores^T[s_k, s_q] = K^T-tile.T @ Q^T -> exp on ACT ->
            # attn@V and ones-row-sum accumulate in PSUM over 16 s_k
            # tiles; normalize by broadcasting 1/l via a K=1 matmul.
            with tc.tile_pool(name="wop", bufs=2) as wop:
                won0 = wop.tile([P, EB, 512], f32, tag="won")
                nc.sync.dma_start(r(won0), r(wo[:, :, 0:512]))  # prefetch for phase 3

                with tc.tile_pool(name="kvl", bufs=2) as kvl, \
                     tc.tile_pool(name="vgp", bufs=2) as vgp, \
                     tc.tile_pool(name="qhp", bufs=3) as qhp, \
                     tc.tile_pool(name="exq", bufs=4) as exq, \
                     tc.tile_pool(name="lsb", bufs=2) as lsb, \
                     tc.tile_pool(name="pscp", bufs=3, space="PSUM") as pscp, \
                     tc.tile_pool(name="psop", bufs=2, space="PSUM") as psop, \
                     tc.tile_pool(name="pslp", bufs=1, space="PSUM") as pslp, \
                     tc.tile_pool(name="psbp", bufs=1, space="PSUM") as psbp:
                    for g in range(G):
                        kt = kvl.tile([P, S], f32, tag="kt")
                        nc.sync.dma_start(r(kt), r(kvT[2 * g * P:(2 * g + 1) * P, :]))
                        vt = kvl.tile([P, S], f32, tag="vt")
                        nc.sync.dma_start(vt, kvT[(2 * g + 1) * P:(2 * g + 2) * P, :])
                        vg = vgp.tile([P, 16, P], bf16, tag="vg")
                        for t in range(16):
                            pv = pscp.tile([P, P], f32, tag="psc")
                            nc.tensor.transpose(pv, vt[:, t * P:(t + 1) * P], ident)
                            nc.vector.tensor_copy(vg[:, t], pv)
                        for hl in range(4):
                            h = 4 * g + hl
                            qh = qhp.tile([P, SQ], f32, tag="qh")
                            nc.sync.dma_start(r(qh), r(qtd[h * P:(h + 1) * P, :]))
                            pso = psop.tile([P, SQ], f32, tag="pso")
                            psl = pslp.tile([1, SQ], f32, tag="psl")
                            exps = [None] * 16

                            def sc(t, qh=qh, kt=kt, exps=exps):
                                p = pscp.tile([P, SQ], f32, tag="psc")
                                nc.tensor.matmul(p, r(kt[:, t * P:(t + 1) * P]),
                                                 r(qh), start=True, stop=True)
                                e = exq.tile([P, SQ], bf16, tag="ex")
                                nc.scalar.activation(e, p, EXP)
                                exps[t] = e

                            sc(0)
                            sc(1)
                            for t in range(16):
                                if t + 2 < 16:
                                    sc(t + 2)
                                e = exps[t]
                                nc.tensor.matmul(pso, vg[:, t], e,
                                                 start=(t == 0), stop=(t == 15))
                                nc.tensor.matmul(psl, onec, e,
                                                 start=(t == 0), stop=(t == 15))
                            li = lsb.tile([1, SQ], f32, tag="li")
                            nc.vector.reciprocal(li, psl)
                            plb = psbp.tile([P, SQ], f32, tag="plb")
                            nc.tensor.matmul(plb, oner, li,
                                             start=True, stop=True)
                            lbs = lsb.tile([P, SQ], f32, tag="lbs")
                            nc.vector.tensor_copy(lbs, plb)
                            nc.vector.tensor_mul(r(OT[:, h]), pso, lbs)

                # ---- Phase 3: output projection. out[s_q, eo] accumulates
                # over 16 head blocks; bias seeded via a K=1 ones matmul.
                with tc.tile_pool(name="obp", bufs=3) as obp, \
                     tc.tile_pool(name="ps3", bufs=2, space="PSUM") as ps3p:
                    for n in range(4):
                        if n == 0:
                            won = won0
                        else:
                            won = wop.tile([P, EB, 512], f32, tag="won")
                            nc.sync.dma_start(r(won), r(wo[:, :, 512 * n:512 * (n + 1)]))
                        for ms in range(4):
                            ps = ps3p.tile([P, 512], f32, tag="ps")
                            nc.tensor.matmul(
                                ps, oner, bo_s[:, 512 * n:512 * (n + 1)],
                                start=True, stop=False)
                            for k in range(EB):
                                nc.tensor.matmul(
                                    ps, r(OT[:, k, ms * P:(ms + 1) * P]),
                                    r(won[:, k]),
                                    start=False, stop=(k == EB - 1))
                            ob = obp.tile([P, 512], f32, tag="ob")
                            nc.vector.tensor_copy(ob, ps)
                            nc.sync.dma_start(
                                out[ms * P:(ms + 1) * P, 512 * n:512 * (n + 1)], ob)

    nc.compile()
    return nc


def _get_nc():
    global _NC
    if _NC is None:
        _NC = _build()
    return _NC


def kernel(x, Wq, bq, Wkv, bkv, Wo, bo):
    from concourse.bass_utils import run_bass_kernel_spmd
    global LAST_RESULT

    x = np.asarray(x, np.float32)
    Wq = np.asarray(Wq, np.float32)
    bq = np.asarray(bq, np.float32)
    Wkv = np.asarray(Wkv, np.float32)
    bkv = np.asarray(bkv, np.float32)
    Wo = np.asarray(Wo, np.float32)
    bo = np.asarray(bo, np.float32)

    nc = _get_nc()
    sc = 1.0 / np.sqrt(E // H)
    # [m, p, b, d]: lhsT tile for Q M-tile m, e-block b
    wq_h = np.ascontiguousarray(
        (Wq * sc).reshape(EB, P, H, P).transpose(2, 1, 0, 3))
    wkv_h = np.ascontiguousarray(Wkv.reshape(EB, P, KV_N).transpose(1, 0, 2))
    wo_h = np.ascontiguousarray(Wo.reshape(EB, P, E).transpose(1, 0, 2))
    bq_h = np.ascontiguousarray((bq * sc).reshape(H, P).T)
    bkv_h = np.ascontiguousarray(bkv.reshape(8, P).T)
    bo_h = np.ascontiguousarray(bo.reshape(1, E))

    in_maps = []
    for c in range(NCORES):
        b, q = divmod(c, 4)
        xT = x[b].T  # [e, s]
        order = [q] + [i for i in range(4) if i != q]
        xtp = np.concatenate([xT[:, 512 * i:512 * (i + 1)] for i in order], axis=1)
        xt_h = np.ascontiguousarray(xtp.reshape(EB, P, S).transpose(1, 0, 2))
        in_maps.append({"xt": xt_h, "wq": wq_h, "wkv": wkv_h, "wo": wo_h,
                        "bq": bq_h, "bkv": bkv_h, "bo": bo_h})

    res = run_bass_kernel_spmd(nc, in_maps, core_ids=list(range(NCORES)),
                               trace=TRACE)
    LAST_RESULT = res

    outf = np.empty((2, S, E), np.float32)
    for c in range(NCORES):
        b, q = divmod(c, 4)
        outf[b, 512 * q:512 * (q + 1), :] = res.results[c]["out"]
    return outf



# revision 9
# speedup vs baseline: 1.2922x; 1.2922x over previous
"""Grouped Query Attention on 8 TRN2 NeuronCores.

Sharding: batch x s_q-quarter (core c -> batch c//4, query rows
[512*(c%4), 512*(c%4+1))). Each core computes the Q projection for its
512 query rows, the full KV projection for its batch (duplicated across
the 4 cores of that batch -- cheaper than collectives), attention for
all 16 heads over its query rows, and the output projection for a
disjoint [512, 2048] slice of the output. Unsharding is concatenation;
no collectives.

v2 (from trace analysis of the f32 baseline, 724us):
- bf16 inputs/weights (host-cast): halves HBM traffic, same PE rate.
- All intermediates (Q^T, K^T, V, attn out) stay SBUF-resident; the
  f32 baseline spilled Q^T/KV^T to DRAM and re-streamed them.
- V is produced directly in [s, d] layout by swapping matmul operand
  roles (lhsT = x^T tile, rhs = Wv block), eliminating 64 PE
  transposes.
- Scores for two s_k tiles share one [128, 1024] PSUM tile, so exp
  runs as 8 wide ACT ops per head instead of 16 (ACT was near the
  per-head PE time).
- Softmax denominators: e-tiles are tree-summed on DVE + Pool (idle
  engines) and reduced with ONE f32r ones-matmul per head, replacing
  16 accumulating [1,512] PE matmuls per head (~55us of PE time).
- 1/sqrt(128) folded into Wq/bq on host; normalization applied by
  broadcasting 1/l via a K=1 f32r matmul.
"""

import numpy as np

E = 2048
S = 2048
P = 128
H = 16
G = 4
SQ = 512          # query rows per core
EB = E // P       # 16 e-blocks (contraction tiles)
NCORES = 8

_NC = None
TRACE = False
LAST_RESULT = None


def _build():
    import concourse.bacc as bacc
    import concourse.mybir as mybir
    import concourse.tile as tile

    f32 = mybir.dt.float32
    f32r = mybir.dt.float32r
    bf16 = mybir.dt.bfloat16
    EXP = mybir.ActivationFunctionType.Exp

    nc = bacc.Bacc("TRN2", target_bir_lowering=False, debug=False,
                   num_devices=NCORES)

    x = nc.declare_dram_parameter("x", [P, 4, EB, 512], bf16, isOutput=False).ap()
    wq = nc.declare_dram_parameter("wq", [H, P, EB, P], bf16, isOutput=False).ap()
    wk = nc.declare_dram_parameter("wk", [P, EB, G * P], bf16, isOutput=False).ap()
    wv = nc.declare_dram_parameter("wv", [P, EB, G * P], bf16, isOutput=False).ap()
    wo = nc.declare_dram_parameter("wo", [P, EB, E], bf16, isOutput=False).ap()
    bq = nc.declare_dram_parameter("bq", [P, H], f32, isOutput=False).ap()
    bk = nc.declare_dram_parameter("bk", [P, G], f32, isOutput=False).ap()
    bvr = nc.declare_dram_parameter("bvr", [1, G * P], f32, isOutput=False).ap()
    bo = nc.declare_dram_parameter("bo", [1, E], f32, isOutput=False).ap()
    out = nc.declare_dram_parameter("out", [SQ, E], f32, isOutput=True).ap()

    def r(ap):
        return ap.bitcast(f32r)

    with tile.TileContext(nc) as tc, \
         nc.allow_low_precision(reason="bf16 intermediates; end-to-end rel-err checked"):
        with tc.tile_pool(name="consts", bufs=1) as cp, \
             tc.tile_pool(name="keep", bufs=1) as kp:
            onec = cp.tile([P, 1], bf16, tag="onec")
            nc.vector.memset(onec, 1.0)
            oner0 = cp.tile([1, P], f32, tag="oner0")
            nc.vector.memset(oner0, 1.0)
            oner = cp.tile([1, P], f32, tag="oner")
            nc.vector.tensor_copy(r(oner), oner0)
            bq_s = cp.tile([P, H], f32, tag="bqs")
            nc.sync.dma_start(bq_s, bq)
            bk_s = cp.tile([P, G], f32, tag="bks")
            nc.sync.dma_start(bk_s, bk)
            bv_s = cp.tile([1, G * P], f32, tag="bvs")
            nc.sync.dma_start(r(bv_s), r(bvr))
            bo_s = cp.tile([1, E], f32, tag="bos")
            nc.sync.dma_start(r(bo_s), r(bo))

            qT = kp.tile([P, H, SQ], bf16, tag="qt")    # Q^T per head block
            kT = kp.tile([P, G, S], bf16, tag="kt")     # K^T per group
            Vs = kp.tile([P, EB, G * P], bf16, tag="vs")  # V[s_tile, 4 groups*128]
            OT = kp.tile([P, H, SQ], bf16, tag="ot")    # normalized attn out

            # ---- Phase 1: projections (PE-bound). x^T arrives in 4
            # column chunks; Q needs only chunk 0 (this core's own
            # rotated s_q quarter), so compute starts after ~2.6MB DMA.
            with tc.tile_pool(name="xp", bufs=1) as xp, \
                 tc.tile_pool(name="wqp", bufs=2) as wqp, \
                 tc.tile_pool(name="wkvp", bufs=1) as wkvp, \
                 tc.tile_pool(name="ps1", bufs=3, space="PSUM") as ps1, \
                 tc.tile_pool(name="ps1v", bufs=3, space="PSUM") as ps1v:
                x4 = []
                for j in range(4):
                    xc = xp.tile([P, EB, 512], bf16, tag=f"x{j}")
                    nc.sync.dma_start(xc, x[:, j])
                    x4.append(xc)
                wk_s = wkvp.tile([P, EB, G * P], bf16, tag="wks")
                nc.sync.dma_start(wk_s, wk)
                wv_s = wkvp.tile([P, EB, G * P], bf16, tag="wvs")
                nc.sync.dma_start(wv_s, wv)

                # Q projection: QT[d, s_q] for 16 head blocks
                for m in range(H):
                    wqm = wqp.tile([P, EB, P], bf16, tag="wqm")
                    nc.sync.dma_start(wqm, wq[m])
                    ps = ps1.tile([P, SQ], f32, tag="ps")
                    for b in range(EB):
                        nc.tensor.matmul(ps, wqm[:, b], x4[0][:, b],
                                         start=(b == 0), stop=(b == EB - 1))
                    nc.vector.tensor_scalar_add(qT[:, m], ps, bq_s[:, m:m + 1])

                # K^T: per group, full (rotated) sequence in 4 chunks
                for g in range(G):
                    for j in range(4):
                        ps = ps1.tile([P, 512], f32, tag="ps")
                        for b in range(EB):
                            nc.tensor.matmul(
                                ps, wk_s[:, b, g * P:(g + 1) * P], x4[j][:, b],
                                start=(b == 0), stop=(b == EB - 1))
                        nc.vector.tensor_scalar_add(
                            kT[:, g, 512 * j:512 * (j + 1)], ps,
                            bk_s[:, g:g + 1])

                # V directly in [s, d] layout: lhsT = x^T tile (e x s),
                # rhs = Wv block (e x 512). Bias seeded via K=1 matmul.
                for t in range(EB):
                    ps = ps1v.tile([P, G * P], f32, tag="psv")
                    nc.tensor.matmul(ps, r(oner), r(bv_s),
                                     start=True, stop=False)
                    j, c = divmod(t, 4)
                    for b in range(EB):
                        nc.tensor.matmul(
                            ps, x4[j][:, b, c * P:(c + 1) * P], wv_s[:, b],
                            start=False, stop=(b == EB - 1))
                    nc.scalar.copy(Vs[:, t], ps)

            # ---- Phase 2: attention. scores^T for two s_k tiles land in
            # one [128,1024] PSUM tile -> one exp -> two attn@V matmuls.
            # Denominator: DVE+Pool tree-sum of e tiles, one f32r
            # ones-matmul, reciprocal, K=1 broadcast matmul.
            with tc.tile_pool(name="wop", bufs=1) as wop:
                wo_s = wop.tile([P, EB, E], bf16, tag="wos")
                nc.sync.dma_start(wo_s, wo)  # prefetch for phase 3

                with tc.tile_pool(name="exq", bufs=5) as exq, \
                     tc.tile_pool(name="accp", bufs=8) as accp, \
                     tc.tile_pool(name="lsb", bufs=2) as lsb, \
                     tc.tile_pool(name="pscp", bufs=2, space="PSUM") as pscp, \
                     tc.tile_pool(name="psop", bufs=2, space="PSUM") as psop, \
                     tc.tile_pool(name="pslp", bufs=1, space="PSUM") as pslp, \
                     tc.tile_pool(name="psbp", bufs=1, space="PSUM") as psbp:
                    for g in range(G):
                        for hl in range(4):
                            h = 4 * g + hl
                            qh = qT[:, h]
                            pso = psop.tile([P, SQ], f32, tag="pso")
                            exps = [None] * 8

                            def sc(i, g=g, qh=qh, exps=exps):
                                # two s_k tiles -> one [128,1024] psum
                                p = pscp.tile([P, 1024], f32, tag="psc")
                                for u in range(2):
                                    t = 2 * i + u
                                    nc.tensor.matmul(
                                        p[:, 512 * u:512 * (u + 1)],
                                        kT[:, g, t * P:(t + 1) * P], qh,
                                        start=True, stop=True)
                                e = exq.tile([P, 1024], bf16, tag="ex")
                                nc.scalar.activation(e, p, EXP)
                                exps[i] = e

                            def av(i, g=g, pso=pso, exps=exps):
                                e = exps[i]
                                for u in range(2):
                                    t = 2 * i + u
                                    nc.tensor.matmul(
                                        pso, Vs[:, t, g * P:(g + 1) * P],
                                        e[:, 512 * u:512 * (u + 1)],
                                        start=(i == 0 and u == 0),
                                        stop=(i == 7 and u == 1))

                            sc(0)
                            sc(1)
                            lvl1 = []
                            for i in range(8):
                                if i + 2 < 8:
                                    sc(i + 2)
                                av(i)
                                if i % 2 == 1:
                                    a = accp.tile([P, 1024], f32, tag="acc")
                                    eng = nc.vector if i == 1 or i == 5 \
                                        else nc.gpsimd
                                    eng.tensor_add(a, exps[i - 1], exps[i])
                                    lvl1.append(a)
                            a01 = accp.tile([P, 1024], f32, tag="acc")
                            nc.gpsimd.tensor_add(a01, lvl1[0], lvl1[1])
                            a23 = accp.tile([P, 1024], f32, tag="acc")
                            nc.gpsimd.tensor_add(a23, lvl1[2], lvl1[3])
                            aall = accp.tile([P, 1024], f32, tag="acc")
                            nc.vector.tensor_add(aall, a01, a23)
                            esum = lsb.tile([P, 512], bf16, tag="esum")
                            nc.vector.tensor_add(esum, aall[:, 0:512],
                                                 aall[:, 512:1024])

                            psl = pslp.tile([1, SQ], f32, tag="psl")
                            nc.tensor.matmul(psl, onec, esum,
                                             start=True, stop=True)
                            li = lsb.tile([1, SQ], f32, tag="li")
                            nc.vector.reciprocal(r(li), psl)
                            plb = psbp.tile([P, SQ], f32, tag="plb")
                            nc.tensor.matmul(plb, r(oner), r(li),
                                             start=True, stop=True)
                            lbs = lsb.tile([P, SQ], f32, tag="lbs")
                            nc.vector.tensor_copy(lbs, plb)
                            nc.vector.tensor_mul(OT[:, h], pso, lbs)

                # ---- Phase 3: output projection. out[s_q, eo] accumulates
                # over 16 head blocks; bias seeded via a K=1 ones matmul.
                with tc.tile_pool(name="obp", bufs=3) as obp, \
                     tc.tile_pool(name="ps3", bufs=2, space="PSUM") as ps3p:
                    for n in range(4):
                        for ms in range(4):
                            ps = ps3p.tile([P, 512], f32, tag="ps")
                            nc.tensor.matmul(
                                ps, r(oner), r(bo_s[:, 512 * n:512 * (n + 1)]),
                                start=True, stop=False)
                            for k in range(EB):
                                nc.tensor.matmul(
                                    ps, OT[:, k, ms * P:(ms + 1) * P],
                                    wo_s[:, k, 512 * n:512 * (n + 1)],
                                    start=False, stop=(k == EB - 1))
                            ob = obp.tile([P, 512], f32, tag="ob")
                            nc.vector.tensor_copy(ob, ps)
                            nc.sync.dma_start(
                                out[ms * P:(ms + 1) * P, 512 * n:512 * (n + 1)], ob)

    nc.compile()
    return nc


def _get_nc():
    global _NC
    if _NC is None:
        _NC = _build()
    return _NC


def kernel(x, Wq, bq, Wkv, bkv, Wo, bo):
    from concourse.bass_utils import run_bass_kernel_spmd
    import ml_dtypes
    global LAST_RESULT

    bft = ml_dtypes.bfloat16
    x = np.asarray(x, np.float32)
    Wq = np.asarray(Wq, np.float32)
    bq = np.asarray(bq, np.float32)
    Wkv = np.asarray(Wkv, np.float32)
    bkv = np.asarray(bkv, np.float32)
    Wo = np.asarray(Wo, np.float32)
    bo = np.asarray(bo, np.float32)

    nc = _get_nc()
    sc = 1.0 / np.sqrt(E // H)
    # [m, p, b, d]: lhsT tile for Q M-tile m, e-block b
    wq_h = np.ascontiguousarray(
        (Wq * sc).reshape(EB, P, H, P).transpose(2, 1, 0, 3)).astype(bft)
    # K / V column split of Wkv ([K0 V0 K1 V1 ...] blocks of 128)
    kcols = np.concatenate(
        [np.arange(g * 2 * P, g * 2 * P + P) for g in range(G)])
    vcols = kcols + P
    wk_h = np.ascontiguousarray(
        Wkv[:, kcols].reshape(EB, P, G * P).transpose(1, 0, 2)).astype(bft)
    wv_h = np.ascontiguousarray(
        Wkv[:, vcols].reshape(EB, P, G * P).transpose(1, 0, 2)).astype(bft)
    wo_h = np.ascontiguousarray(
        Wo.reshape(EB, P, E).transpose(1, 0, 2)).astype(bft)
    bq_h = np.ascontiguousarray((bq * sc).reshape(H, P).T)
    bk_h = np.ascontiguousarray(bkv[kcols].reshape(G, P).T)
    bv_h = np.ascontiguousarray(bkv[vcols].reshape(1, G * P))
    bo_h = np.ascontiguousarray(bo.reshape(1, E))

    in_maps = []
    for c in range(NCORES):
        b, q = divmod(c, 4)
        xT = x[b].T.astype(bft)  # [e, s]
        order = [q] + [i for i in range(4) if i != q]
        # [p, chunk, b, 512] with this core's s_q quarter as chunk 0
        xt_h = np.ascontiguousarray(
            np.stack([xT[:, 512 * i:512 * (i + 1)].reshape(EB, P, 512)
                      for i in order], axis=0).transpose(2, 0, 1, 3))
        in_maps.append({"x": xt_h, "wq": wq_h, "wk": wk_h, "wv": wv_h,
                        "wo": wo_h, "bq": bq_h, "bk": bk_h, "bvr": bv_h,
                        "bo": bo_h})

    res = run_bass_kernel_spmd(nc, in_maps, core_ids=list(range(NCORES)),
                               trace=TRACE)
    LAST_RESULT = res

    outf = np.empty((2, S, E), np.float32)
    for c in range(NCORES):
        b, q = divmod(c, 4)
        outf[b, 512 * q:512 * (q + 1), :] = res.results[c]["out"]
    return outf


# revision 22
# speedup vs baseline: 1.3626x; 1.0545x over previous
"""Grouped Query Attention on 8 TRN2 NeuronCores.

Sharding: batch x s_q-quarter (core c -> batch c//4, query rows
[512*(c%4), 512*(c%4+1))). Each core computes the Q projection for its
512 query rows, the full KV projection for its batch (duplicated across
the 4 cores of that batch -- cheaper than collectives), attention for
all 16 heads over its query rows, and the output projection for a
disjoint [512, 2048] slice of the output. Unsharding is concatenation;
no collectives.

v2 (from trace analysis of the f32 baseline, 724us):
- bf16 inputs/weights (host-cast): halves HBM traffic, same PE rate.
- All intermediates (Q^T, K^T, V, attn out) stay SBUF-resident; the
  f32 baseline spilled Q^T/KV^T to DRAM and re-streamed them.
- V is produced directly in [s, d] layout by swapping matmul operand
  roles (lhsT = x^T tile, rhs = Wv block), eliminating 64 PE
  transposes.
- Scores for two s_k tiles share one [128, 1024] PSUM tile, so exp
  runs as 8 wide ACT ops per head instead of 16 (ACT was near the
  per-head PE time).
- Softmax denominators: e-tiles are tree-summed on DVE + Pool (idle
  engines) and reduced with ONE f32r ones-matmul per head, replacing
  16 accumulating [1,512] PE matmuls per head (~55us of PE time).
- 1/sqrt(128) folded into Wq/bq on host; normalization applied by
  broadcasting 1/l via a K=1 f32r matmul.
"""

import numpy as np

E = 2048
S = 2048
P = 128
H = 16
G = 4
SQ = 512          # query rows per core
EB = E // P       # 16 e-blocks (contraction tiles)
NCORES = 8

_NC = None
TRACE = False
LAST_RESULT = None


def _build():
    import concourse.bacc as bacc
    import concourse.mybir as mybir
    import concourse.tile as tile

    f32 = mybir.dt.float32
    f32r = mybir.dt.float32r
    bf16 = mybir.dt.bfloat16
    EXP = mybir.ActivationFunctionType.Exp

    nc = bacc.Bacc("TRN2", target_bir_lowering=False, debug=False,
                   num_devices=NCORES)

    x = nc.declare_dram_parameter("x", [P, 4, EB, 512], bf16, isOutput=False).ap()
    wq = nc.declare_dram_parameter("wq", [H, P, EB, P], bf16, isOutput=False).ap()
    wk = nc.declare_dram_parameter("wk", [P, EB, G * P], bf16, isOutput=False).ap()
    wv = nc.declare_dram_parameter("wv", [P, EB, G * P], bf16, isOutput=False).ap()
    wo = nc.declare_dram_parameter("wo", [P, EB, E], bf16, isOutput=False).ap()
    bq = nc.declare_dram_parameter("bq", [P, H], f32, isOutput=False).ap()
    bk = nc.declare_dram_parameter("bk", [P, G], f32, isOutput=False).ap()
    bvr = nc.declare_dram_parameter("bvr", [1, G * P], f32, isOutput=False).ap()
    bo = nc.declare_dram_parameter("bo", [1, E], f32, isOutput=False).ap()
    oc4d = nc.declare_dram_parameter("oc4d", [P, 4, 4], bf16, isOutput=False).ap()
    selrd = nc.declare_dram_parameter("selrd", [4, 4, P], f32, isOutput=False).ap()
    out = nc.declare_dram_parameter("out", [SQ, E], f32, isOutput=True).ap()

    def r(ap):
        return ap.bitcast(f32r)

    with tile.TileContext(nc) as tc, \
         nc.allow_low_precision(reason="bf16 intermediates; end-to-end rel-err checked"):
        with tc.tile_pool(name="consts", bufs=1) as cp, \
             tc.tile_pool(name="keep", bufs=1) as kp:
            onec = cp.tile([P, 1], bf16, tag="onec")
            nc.vector.memset(onec, 1.0)
            oner0 = cp.tile([1, P], f32, tag="oner0")
            nc.vector.memset(oner0, 1.0)
            oner = cp.tile([1, P], f32, tag="oner")
            nc.vector.tensor_copy(r(oner), oner0)
            # indicator constants for batched softmax-denominator rows
            # (host-prepared): oc4[:, hl] is [128,4] with column hl
            # all-ones (ones-matmul lhsT -> row hl of a [4,512] psum
            # tile); selr[:, hl] is [4,128] with row hl all-ones
            # (broadcast-back lhsT).
            oc4 = cp.tile([P, 4, 4], bf16, tag="oc4")
            nc.sync.dma_start(oc4, oc4d)
            selr = cp.tile([4, 4, P], f32, tag="selr")
            nc.sync.dma_start(r(selr), r(selrd))
            bq_s = cp.tile([P, H], f32, tag="bqs")
            nc.sync.dma_start(bq_s, bq)
            bk_s = cp.tile([P, G], f32, tag="bks")
            nc.sync.dma_start(bk_s, bk)
            bv_s = cp.tile([1, G * P], f32, tag="bvs")
            nc.sync.dma_start(r(bv_s), r(bvr))
            bo_s = cp.tile([1, E], f32, tag="bos")
            nc.sync.dma_start(r(bo_s), r(bo))

            qT = kp.tile([P, H, SQ], bf16, tag="qt")    # Q^T per head block
            kT = kp.tile([P, G, S], bf16, tag="kt")     # K^T per group
            Vs = kp.tile([P, EB, G * P], bf16, tag="vs")  # V[s_tile, 4 groups*128]
            OT = kp.tile([P, H, SQ], bf16, tag="ot")    # normalized attn out

            # ---- Phase 1: projections (PE-bound). x^T arrives in 4
            # column chunks; Q needs only chunk 0 (this core's own
            # rotated s_q quarter), so compute starts after ~2.6MB DMA.
            # DMA issue order matters: the first Q matmul must not queue
            # behind the other 12.6MB, so x1-3/wk/wv are issued from
            # inside the Q loop.
            with tc.tile_pool(name="xp", bufs=1) as xp, \
                 tc.tile_pool(name="wqp", bufs=3) as wqp, \
                 tc.tile_pool(name="wkvp", bufs=1) as wkvp, \
                 tc.tile_pool(name="ps1", bufs=3, space="PSUM") as ps1, \
                 tc.tile_pool(name="ps1v", bufs=3, space="PSUM") as ps1v:
                x4 = [xp.tile([P, EB, 512], bf16, tag=f"x{j}", name=f"x{j}")
                      for j in range(4)]
                nc.sync.dma_start(x4[0], x[:, 0])
                wk_s = wkvp.tile([P, EB, G * P], bf16, tag="wks")
                wv_s = wkvp.tile([P, EB, G * P], bf16, tag="wvs")

                # Q projection: QT[d, s_q] for 16 head blocks
                wqts = [wqp.tile([P, EB, P], bf16, tag="wqm", name="wqm")
                        for m in range(3)]
                for m in range(3):
                    nc.sync.dma_start(wqts[m], wq[m])
                for m in range(H):
                    wqm = wqts[m]
                    if m + 3 < H:
                        w_next = wqp.tile([P, EB, P], bf16, tag="wqm")
                        nc.sync.dma_start(w_next, wq[m + 3])
                        wqts.append(w_next)
                    if m in (4, 8, 12):
                        nc.sync.dma_start(x4[m // 4], x[:, m // 4])
                    if m == 13:
                        nc.sync.dma_start(wk_s, wk)
                    if m == 14:
                        nc.sync.dma_start(wv_s, wv)
                    ps = ps1.tile([P, SQ], f32, tag="ps")
                    for b in range(EB):
                        nc.tensor.matmul(ps, wqm[:, b], x4[0][:, b],
                                         start=(b == 0), stop=(b == EB - 1))
                    nc.vector.tensor_scalar_add(qT[:, m], ps, bq_s[:, m:m + 1])

                # K^T: per group, full (rotated) sequence in 4 chunks
                for g in range(G):
                    for j in range(4):
                        ps = ps1.tile([P, 512], f32, tag="ps")
                        for b in range(EB):
                            nc.tensor.matmul(
                                ps, wk_s[:, b, g * P:(g + 1) * P], x4[j][:, b],
                                start=(b == 0), stop=(b == EB - 1))
                        nc.vector.tensor_scalar_add(
                            kT[:, g, 512 * j:512 * (j + 1)], ps,
                            bk_s[:, g:g + 1])

                # V directly in [s, d] layout: lhsT = x^T tile (e x s),
                # rhs = Wv block (e x 512). Bias seeded via K=1 matmul.
                for t in range(EB):
                    ps = ps1v.tile([P, G * P], f32, tag="psv")
                    nc.tensor.matmul(ps, r(oner), r(bv_s),
                                     start=True, stop=False)
                    j, c = divmod(t, 4)
                    for b in range(EB):
                        nc.tensor.matmul(
                            ps, x4[j][:, b, c * P:(c + 1) * P], wv_s[:, b],
                            start=False, stop=(b == EB - 1))
                    nc.scalar.copy(Vs[:, t], ps)

            # ---- Phase 2: attention. scores^T for two s_k tiles land in
            # one [128,1024] PSUM tile -> one exp -> two attn@V matmuls.
            # Denominator: DVE+Pool tree-sum of e tiles, one f32r
            # ones-matmul, reciprocal, K=1 broadcast matmul.
            with tc.tile_pool(name="wop", bufs=1) as wop:
                wo_s = wop.tile([P, EB, E], bf16, tag="wos")
                nc.sync.dma_start(wo_s, wo)  # prefetch for phase 3

                with tc.tile_pool(name="exq", bufs=5) as exq, \
                     tc.tile_pool(name="accp", bufs=8) as accp, \
                     tc.tile_pool(name="lsb", bufs=2) as lsb, \
                     tc.tile_pool(name="psfp", bufs=5) as psfp, \
                     tc.tile_pool(name="pscp", bufs=2, space="PSUM") as pscp, \
                     tc.tile_pool(name="psop", bufs=2, space="PSUM") as psop, \
                     tc.tile_pool(name="pslp", bufs=1, space="PSUM") as pslp, \
                     tc.tile_pool(name="psbp", bufs=1, space="PSUM") as psbp:
                    # pipelined per-head state
                    pend = [None]        # (hl, esum, psl4) awaiting psl matmul
                    grp = [None]         # (psl4, li4, psofs) of group being normalized

                    def emit_psl():
                        hl_p, esum_p, psl4_p = pend[0]
                        nc.tensor.matmul(psl4_p, oc4[:, hl_p], esum_p,
                                         start=(hl_p == 0), stop=(hl_p == 3))
                        pend[0] = None

                    def emit_group_norm():
                        # recip + broadcast + normalize for a finished group
                        psl4_p, li4, psofs = grp[0]
                        nc.vector.reciprocal(r(li4), psl4_p)
                        for hl_p, (h_p, psof) in enumerate(psofs):
                            plb = psbp.tile([P, SQ], f32, tag="plb")
                            nc.tensor.matmul(plb, r(selr[:, hl_p]),
                                             r(li4),
                                             start=True, stop=True)
                            lbs = lsb.tile([P, SQ], f32, tag="lbs")
                            nc.vector.tensor_copy(lbs, plb)
                            nc.vector.tensor_mul(OT[:, h_p], psof, lbs)
                        grp[0] = None

                    for g in range(G):
                        psl4 = pslp.tile([4, SQ], f32, tag="psl4")
                        li4 = lsb.tile([4, SQ], f32, tag="li4")
                        psofs = []
                        for hl in range(4):
                            h = 4 * g + hl
                            qh = qT[:, h]
                            pso = psop.tile([P, SQ], f32, tag="pso")
                            exps = [None] * 8

                            def sc(i, g=g, qh=qh, exps=exps):
                                # two s_k tiles -> one [128,1024] psum
                                p = pscp.tile([P, 1024], f32, tag="psc")
                                for u in range(2):
                                    t = 2 * i + u
                                    nc.tensor.matmul(
                                        p[:, 512 * u:512 * (u + 1)],
                                        kT[:, g, t * P:(t + 1) * P], qh,
                                        start=True, stop=True)
                                e = exq.tile([P, 1024], bf16, tag="ex")
                                nc.scalar.activation(e, p, EXP)
                                exps[i] = e

                            def av(i, g=g, pso=pso, exps=exps):
                                e = exps[i]
                                for u in range(2):
                                    t = 2 * i + u
                                    nc.tensor.matmul(
                                        pso, Vs[:, t, g * P:(g + 1) * P],
                                        e[:, 512 * u:512 * (u + 1)],
                                        start=(i == 0 and u == 0),
                                        stop=(i == 7 and u == 1))

                            sc(0)
                            sc(1)
                            lvl1 = []
                            for i in range(8):
                                if i + 2 < 8:
                                    sc(i + 2)
                                if i == 2 and pend[0] is not None:
                                    emit_psl()
                                if i == 4 and grp[0] is not None:
                                    emit_group_norm()
                                av(i)
                                if i % 2 == 1:
                                    a = accp.tile([P, 1024], bf16, tag="acc")
                                    eng = nc.vector if i in (1, 5) \
                                        else nc.gpsimd
                                    eng.tensor_add(a, exps[i - 1], exps[i])
                                    lvl1.append(a)
                            a01 = accp.tile([P, 1024], bf16, tag="acc")
                            nc.gpsimd.tensor_add(a01, lvl1[0], lvl1[1])
                            a23 = accp.tile([P, 1024], bf16, tag="acc")
                            nc.gpsimd.tensor_add(a23, lvl1[2], lvl1[3])
                            aall = accp.tile([P, 1024], bf16, tag="acc")
                            nc.vector.tensor_add(aall, a01, a23)
                            esum = lsb.tile([P, 512], bf16, tag="esum")
                            nc.vector.tensor_add(esum, aall[:, 0:512],
                                                 aall[:, 512:1024])
                            # spill pso to SBUF (frees PSUM for next head)
                            psof = psfp.tile([P, SQ], f32, tag="psof")
                            nc.vector.tensor_copy(psof, pso)
                            pend[0] = (hl, esum, psl4)
                            psofs.append((h, psof))
                        grp[0] = (psl4, li4, psofs)
                    emit_psl()
                    emit_group_norm()

                # ---- Phase 3: output projection. out[s_q, eo] accumulates
                # over 16 head blocks; bias seeded via a K=1 ones matmul.
                with tc.tile_pool(name="obp", bufs=3) as obp, \
                     tc.tile_pool(name="ps3", bufs=2, space="PSUM") as ps3p:
                    for n in range(4):
                        for ms in range(4):
                            ps = ps3p.tile([P, 512], f32, tag="ps")
                            nc.tensor.matmul(
                                ps, r(oner), r(bo_s[:, 512 * n:512 * (n + 1)]),
                                start=True, stop=False)
                            for k in range(EB):
                                nc.tensor.matmul(
                                    ps, OT[:, k, ms * P:(ms + 1) * P],
                                    wo_s[:, k, 512 * n:512 * (n + 1)],
                                    start=False, stop=(k == EB - 1))
                            ob = obp.tile([P, 512], f32, tag="ob")
                            nc.vector.tensor_copy(ob, ps)
                            nc.sync.dma_start(
                                out[ms * P:(ms + 1) * P, 512 * n:512 * (n + 1)], ob)

    nc.compile()
    return nc


def _get_nc():
    global _NC
    if _NC is None:
        _NC = _build()
    return _NC


def kernel(x, Wq, bq, Wkv, bkv, Wo, bo):
    from concourse.bass_utils import run_bass_kernel_spmd
    import ml_dtypes
    global LAST_RESULT

    bft = ml_dtypes.bfloat16
    x = np.asarray(x, np.float32)
    Wq = np.asarray(Wq, np.float32)
    bq = np.asarray(bq, np.float32)
    Wkv = np.asarray(Wkv, np.float32)
    bkv = np.asarray(bkv, np.float32)
    Wo = np.asarray(Wo, np.float32)
    bo = np.asarray(bo, np.float32)

    nc = _get_nc()
    sc = 1.0 / np.sqrt(E // H)
    # [m, p, b, d]: lhsT tile for Q M-tile m, e-block b
    wq_h = np.ascontiguousarray(
        (Wq * sc).reshape(EB, P, H, P).transpose(2, 1, 0, 3)).astype(bft)
    # K / V column split of Wkv ([K0 V0 K1 V1 ...] blocks of 128)
    kcols = np.concatenate(
        [np.arange(g * 2 * P, g * 2 * P + P) for g in range(G)])
    vcols = kcols + P
    wk_h = np.ascontiguousarray(
        Wkv[:, kcols].reshape(EB, P, G * P).transpose(1, 0, 2)).astype(bft)
    wv_h = np.ascontiguousarray(
        Wkv[:, vcols].reshape(EB, P, G * P).transpose(1, 0, 2)).astype(bft)
    wo_h = np.ascontiguousarray(
        Wo.reshape(EB, P, E).transpose(1, 0, 2)).astype(bft)
    bq_h = np.ascontiguousarray((bq * sc).reshape(H, P).T)
    bk_h = np.ascontiguousarray(bkv[kcols].reshape(G, P).T)
    bv_h = np.ascontiguousarray(bkv[vcols].reshape(1, G * P))
    bo_h = np.ascontiguousarray(bo.reshape(1, E))
    oc4_h = np.ascontiguousarray(
        np.broadcast_to(np.eye(4, dtype=np.float32), (P, 4, 4))).astype(bft)
    selr_h = np.ascontiguousarray(
        np.repeat(np.eye(4, dtype=np.float32), P, axis=1).reshape(4, 4, P))

    in_maps = []
    for c in range(NCORES):
        b, q = divmod(c, 4)
        xT = x[b].T.astype(bft)  # [e, s]
        order = [q] + [i for i in range(4) if i != q]
        # [p, chunk, b, 512] with this core's s_q quarter as chunk 0
        xt_h = np.ascontiguousarray(
            np.stack([xT[:, 512 * i:512 * (i + 1)].reshape(EB, P, 512)
                      for i in order], axis=0).transpose(2, 0, 1, 3))
        in_maps.append({"x": xt_h, "wq": wq_h, "wk": wk_h, "wv": wv_h,
                        "wo": wo_h, "bq": bq_h, "bk": bk_h, "bvr": bv_h,
                        "bo": bo_h, "oc4d": oc4_h, "selrd": selr_h})

    res = run_bass_kernel_spmd(nc, in_maps, core_ids=list(range(NCORES)),
                               trace=TRACE)
    LAST_RESULT = res

    outf = np.empty((2, S, E), np.float32)
    for c in range(NCORES):
        b, q = divmod(c, 4)
        outf[b, 512 * q:512 * (q + 1), :] = res.results[c]["out"]
    return outf


# revision 26
# speedup vs baseline: 1.5834x; 1.1620x over previous
"""Grouped Query Attention on 8 TRN2 NeuronCores.

Sharding: batch x s_q-quarter (core c -> batch c//4, query rows
[512*(c%4), 512*(c%4+1))). Each core computes the Q projection for its
512 query rows, the full KV projection for its batch (duplicated across
the 4 cores of that batch -- cheaper than collectives), attention for
all 16 heads over its query rows, and the output projection for a
disjoint [512, 2048] slice of the output. Unsharding is concatenation;
no collectives.

v2 (from trace analysis of the f32 baseline, 724us):
- bf16 inputs/weights (host-cast): halves HBM traffic, same PE rate.
- All intermediates (Q^T, K^T, V, attn out) stay SBUF-resident; the
  f32 baseline spilled Q^T/KV^T to DRAM and re-streamed them.
- V is produced directly in [s, d] layout by swapping matmul operand
  roles (lhsT = x^T tile, rhs = Wv block), eliminating 64 PE
  transposes.
- Scores for two s_k tiles share one [128, 1024] PSUM tile, so exp
  runs as 8 wide ACT ops per head instead of 16 (ACT was near the
  per-head PE time).
- Softmax denominators: e-tiles are tree-summed on DVE + Pool (idle
  engines) and reduced with ONE f32r ones-matmul per head, replacing
  16 accumulating [1,512] PE matmuls per head (~55us of PE time).
- 1/sqrt(128) folded into Wq/bq on host; normalization applied by
  broadcasting 1/l via a K=1 f32r matmul.
"""

import numpy as np

E = 2048
S = 2048
P = 128
H = 16
G = 4
SQ = 512          # query rows per core
EB = E // P       # 16 e-blocks (contraction tiles)
NCORES = 8

_NC = None
TRACE = False
LAST_RESULT = None


def _build():
    import concourse.bacc as bacc
    import concourse.mybir as mybir
    import concourse.tile as tile

    f32 = mybir.dt.float32
    f32r = mybir.dt.float32r
    bf16 = mybir.dt.bfloat16
    EXP = mybir.ActivationFunctionType.Exp

    nc = bacc.Bacc("TRN2", target_bir_lowering=False, debug=False,
                   num_devices=NCORES)

    x = nc.declare_dram_parameter("x", [P, 4, EB, 512], bf16, isOutput=False).ap()
    wq = nc.declare_dram_parameter("wq", [H, P, EB, P], bf16, isOutput=False).ap()
    wk = nc.declare_dram_parameter("wk", [P, EB, G * P], bf16, isOutput=False).ap()
    wv = nc.declare_dram_parameter("wv", [P, EB, G * P], bf16, isOutput=False).ap()
    wo = nc.declare_dram_parameter("wo", [P, EB, E], bf16, isOutput=False).ap()
    bq = nc.declare_dram_parameter("bq", [P, H], f32, isOutput=False).ap()
    bk = nc.declare_dram_parameter("bk", [P, G], f32, isOutput=False).ap()
    bvr = nc.declare_dram_parameter("bvr", [1, G * P], f32, isOutput=False).ap()
    bo = nc.declare_dram_parameter("bo", [1, E], f32, isOutput=False).ap()
    oc4d = nc.declare_dram_parameter("oc4d", [P, 4, 4], bf16, isOutput=False).ap()
    selrd = nc.declare_dram_parameter("selrd", [4, 4, P], f32, isOutput=False).ap()
    out = nc.declare_dram_parameter("out", [SQ, E], f32, isOutput=True).ap()

    def r(ap):
        return ap.bitcast(f32r)

    with tile.TileContext(nc) as tc, \
         nc.allow_low_precision(reason="bf16 intermediates; end-to-end rel-err checked"):
        with tc.tile_pool(name="consts", bufs=1) as cp, \
             tc.tile_pool(name="keep", bufs=1) as kp:
            onec = cp.tile([P, 1], bf16, tag="onec")
            nc.vector.memset(onec, 1.0)
            oner0 = cp.tile([1, P], f32, tag="oner0")
            nc.vector.memset(oner0, 1.0)
            oner = cp.tile([1, P], f32, tag="oner")
            nc.vector.tensor_copy(r(oner), oner0)
            # indicator constants for batched softmax-denominator rows
            # (host-prepared): oc4[:, hl] is [128,4] with column hl
            # all-ones (ones-matmul lhsT -> row hl of a [4,512] psum
            # tile); selr[:, hl] is [4,128] with row hl all-ones
            # (broadcast-back lhsT).
            oc4 = cp.tile([P, 4, 4], bf16, tag="oc4")
            nc.sync.dma_start(oc4, oc4d)
            selr = cp.tile([4, 4, P], f32, tag="selr")
            nc.sync.dma_start(r(selr), r(selrd))
            bq_s = cp.tile([P, H], f32, tag="bqs")
            nc.sync.dma_start(bq_s, bq)
            bk_s = cp.tile([P, G], f32, tag="bks")
            nc.sync.dma_start(bk_s, bk)
            bv_s = cp.tile([1, G * P], f32, tag="bvs")
            nc.sync.dma_start(r(bv_s), r(bvr))
            bo_s = cp.tile([1, E], f32, tag="bos")
            nc.sync.dma_start(r(bo_s), r(bo))

            qT = kp.tile([P, H, SQ], bf16, tag="qt")    # Q^T per head block
            kT = kp.tile([P, G, S], bf16, tag="kt")     # K^T per group
            Vs = kp.tile([P, EB, G * P], bf16, tag="vs")  # V[s_tile, 4 groups*128]
            OT = kp.tile([P, H, SQ], bf16, tag="ot")    # normalized attn out

            # ---- Phase 1: projections (PE-bound). x^T arrives in 4
            # column chunks; Q needs only chunk 0 (this core's own
            # rotated s_q quarter), so compute starts after ~2.6MB DMA.
            # DMA issue order matters: the first Q matmul must not queue
            # behind the other 12.6MB, so x1-3/wk/wv are issued from
            # inside the Q loop.
            with tc.tile_pool(name="xp", bufs=1) as xp, \
                 tc.tile_pool(name="wqp", bufs=3) as wqp, \
                 tc.tile_pool(name="wkvp", bufs=1) as wkvp, \
                 tc.tile_pool(name="ps1", bufs=3, space="PSUM") as ps1, \
                 tc.tile_pool(name="ps1v", bufs=3, space="PSUM") as ps1v:
                x4 = [xp.tile([P, EB, 512], bf16, tag=f"x{j}", name=f"x{j}")
                      for j in range(4)]
                nc.sync.dma_start(x4[0], x[:, 0])
                wk_s = wkvp.tile([P, EB, G * P], bf16, tag="wks")
                wv_s = wkvp.tile([P, EB, G * P], bf16, tag="wvs")

                # Q projection: QT[d, s_q] for 16 head blocks
                wqts = [wqp.tile([P, EB, P], bf16, tag="wqm", name="wqm")
                        for m in range(3)]
                for m in range(3):
                    nc.sync.dma_start(wqts[m], wq[m])
                for m in range(H):
                    wqm = wqts[m]
                    if m + 3 < H:
                        w_next = wqp.tile([P, EB, P], bf16, tag="wqm")
                        nc.sync.dma_start(w_next, wq[m + 3])
                        wqts.append(w_next)
                    if m in (4, 8, 12):
                        nc.sync.dma_start(x4[m // 4], x[:, m // 4])
                    if m == 13:
                        nc.sync.dma_start(wk_s, wk)
                    if m == 14:
                        nc.sync.dma_start(wv_s, wv)
                    ps = ps1.tile([P, SQ], f32, tag="ps")
                    for b in range(EB):
                        nc.tensor.matmul(ps, wqm[:, b], x4[0][:, b],
                                         start=(b == 0), stop=(b == EB - 1))
                    nc.vector.tensor_scalar_add(qT[:, m], ps, bq_s[:, m:m + 1])

                # K^T: per group, full (rotated) sequence in 4 chunks
                for g in range(G):
                    for j in range(4):
                        ps = ps1.tile([P, 512], f32, tag="ps")
                        for b in range(EB):
                            nc.tensor.matmul(
                                ps, wk_s[:, b, g * P:(g + 1) * P], x4[j][:, b],
                                start=(b == 0), stop=(b == EB - 1))
                        nc.vector.tensor_scalar_add(
                            kT[:, g, 512 * j:512 * (j + 1)], ps,
                            bk_s[:, g:g + 1])

                # V directly in [s, d] layout: lhsT = x^T tile (e x s),
                # rhs = Wv block (e x 512). Bias seeded via K=1 matmul.
                for t in range(EB):
                    ps = ps1v.tile([P, G * P], f32, tag="psv")
                    nc.tensor.matmul(ps, r(oner), r(bv_s),
                                     start=True, stop=False)
                    j, c = divmod(t, 4)
                    for b in range(EB):
                        nc.tensor.matmul(
                            ps, x4[j][:, b, c * P:(c + 1) * P], wv_s[:, b],
                            start=False, stop=(b == EB - 1))
                    nc.scalar.copy(Vs[:, t], ps)

            # ---- Phase 2: attention. scores^T for two s_k tiles land in
            # one [128,1024] PSUM tile -> one exp -> two attn@V matmuls.
            # Denominator: DVE+Pool tree-sum of e tiles, one f32r
            # ones-matmul, reciprocal, K=1 broadcast matmul.
            with tc.tile_pool(name="wop", bufs=1) as wop:
                wo_s = wop.tile([P, EB, E], bf16, tag="wos")
                nc.sync.dma_start(wo_s, wo)  # prefetch for phase 3

                with tc.tile_pool(name="exq", bufs=5) as exq, \
                     tc.tile_pool(name="accp", bufs=8) as accp, \
                     tc.tile_pool(name="lsb", bufs=2) as lsb, \
                     tc.tile_pool(name="psfp", bufs=5) as psfp, \
                     tc.tile_pool(name="pscp", bufs=2, space="PSUM") as pscp, \
                     tc.tile_pool(name="psop", bufs=2, space="PSUM") as psop, \
                     tc.tile_pool(name="pslp", bufs=1, space="PSUM") as pslp, \
                     tc.tile_pool(name="psbp", bufs=1, space="PSUM") as psbp:
                    # pipelined per-head state
                    pend = [None]   # (hl, a01, a23, psl4) awaiting psl matmuls
                    recq = []       # [(psl4, li4)] groups awaiting reciprocal
                    finq = []       # [(h, hl, psof, li4)] awaiting plb+mul

                    def emit_psl():
                        # denominator rows: 4 accumulating [4,512] matmuls
                        # straight off the two level-2 tree partials
                        hl_p, a01_p, a23_p, psl4_p = pend[0]
                        for k, rhs in enumerate(
                                (a01_p[:, 0:512], a01_p[:, 512:1024],
                                 a23_p[:, 0:512], a23_p[:, 512:1024])):
                            nc.tensor.matmul(psl4_p, oc4[:, hl_p], rhs,
                                             start=(hl_p == 0 and k == 0),
                                             stop=(hl_p == 3 and k == 3))
                        pend[0] = None

                    def emit_recip():
                        psl4_p, li4, staged = recq.pop(0)
                        nc.vector.reciprocal(r(li4), psl4_p)
                        finq.extend(staged)

                    def emit_fin():
                        h_p, hl_p, psof, li4 = finq.pop(0)
                        plb = psbp.tile([P, SQ], f32, tag="plb")
                        nc.tensor.matmul(plb, r(selr[:, hl_p]), r(li4),
                                         start=True, stop=True)
                        lbs = lsb.tile([P, SQ], f32, tag="lbs")
                        nc.vector.tensor_copy(lbs, plb)
                        nc.gpsimd.tensor_mul(OT[:, h_p], psof, lbs)

                    for g in range(G):
                        psl4 = pslp.tile([4, SQ], f32, tag="psl4")
                        li4 = lsb.tile([4, SQ], f32, tag="li4")
                        grp_stage = []
                        for hl in range(4):
                            h = 4 * g + hl
                            qh = qT[:, h]
                            pso = psop.tile([P, SQ], f32, tag="pso")
                            exps = [None] * 8

                            def sc(i, g=g, qh=qh, exps=exps):
                                # two s_k tiles -> one [128,1024] psum
                                p = pscp.tile([P, 1024], f32, tag="psc")
                                for u in range(2):
                                    t = 2 * i + u
                                    nc.tensor.matmul(
                                        p[:, 512 * u:512 * (u + 1)],
                                        kT[:, g, t * P:(t + 1) * P], qh,
                                        start=True, stop=True)
                                e = exq.tile([P, 1024], bf16, tag="ex")
                                nc.scalar.activation(e, p, EXP)
                                exps[i] = e

                            def av(i, g=g, pso=pso, exps=exps):
                                e = exps[i]
                                for u in range(2):
                                    t = 2 * i + u
                                    nc.tensor.matmul(
                                        pso, Vs[:, t, g * P:(g + 1) * P],
                                        e[:, 512 * u:512 * (u + 1)],
                                        start=(i == 0 and u == 0),
                                        stop=(i == 7 and u == 1))

                            sc(0)
                            sc(1)
                            lvl1 = []
                            for i in range(8):
                                if i + 2 < 8:
                                    sc(i + 2)
                                if i == 1 and pend[0] is not None:
                                    emit_psl()
                                if i == 3 and recq:
                                    emit_recip()
                                if i in (5, 7) and finq:
                                    emit_fin()
                                av(i)
                                if i % 2 == 1:
                                    a = accp.tile([P, 1024], bf16, tag="acc")
                                    # early tiles on the slow engine, late
                                    # tiles on DVE (short tail chain)
                                    eng = nc.gpsimd if i in (1, 3) \
                                        else nc.vector
                                    eng.tensor_add(a, exps[i - 1], exps[i])
                                    lvl1.append(a)
                            a01 = accp.tile([P, 1024], bf16, tag="acc")
                            nc.gpsimd.tensor_add(a01, lvl1[0], lvl1[1])
                            a23 = accp.tile([P, 1024], bf16, tag="acc")
                            nc.vector.tensor_add(a23, lvl1[2], lvl1[3])
                            # spill pso to SBUF (frees PSUM for next head)
                            psof = psfp.tile([P, SQ], f32, tag="psof")
                            nc.vector.tensor_copy(psof, pso)
                            pend[0] = (hl, a01, a23, psl4)
                            grp_stage.append((h, hl, psof, li4))
                        recq.append((psl4, li4, grp_stage))
                    emit_psl()
                    while recq:
                        emit_recip()
                    while finq:
                        emit_fin()

                # ---- Phase 3: output projection. out[s_q, eo] accumulates
                # over 16 head blocks; bias seeded via a K=1 ones matmul.
                with tc.tile_pool(name="obp", bufs=3) as obp, \
                     tc.tile_pool(name="ps3", bufs=2, space="PSUM") as ps3p:
                    for n in range(4):
                        for ms in range(4):
                            ps = ps3p.tile([P, 512], f32, tag="ps")
                            nc.tensor.matmul(
                                ps, r(oner), r(bo_s[:, 512 * n:512 * (n + 1)]),
                                start=True, stop=False)
                            for k in range(EB):
                                nc.tensor.matmul(
                                    ps, OT[:, k, ms * P:(ms + 1) * P],
                                    wo_s[:, k, 512 * n:512 * (n + 1)],
                                    start=False, stop=(k == EB - 1))
                            ob = obp.tile([P, 512], f32, tag="ob")
                            nc.vector.tensor_copy(ob, ps)
                            nc.sync.dma_start(
                                out[ms * P:(ms + 1) * P, 512 * n:512 * (n + 1)], ob)

    nc.compile()
    return nc


def _get_nc():
    global _NC
    if _NC is None:
        _NC = _build()
    return _NC


def kernel(x, Wq, bq, Wkv, bkv, Wo, bo):
    from concourse.bass_utils import run_bass_kernel_spmd
    import ml_dtypes
    global LAST_RESULT

    bft = ml_dtypes.bfloat16
    x = np.asarray(x, np.float32)
    Wq = np.asarray(Wq, np.float32)
    bq = np.asarray(bq, np.float32)
    Wkv = np.asarray(Wkv, np.float32)
    bkv = np.asarray(bkv, np.float32)
    Wo = np.asarray(Wo, np.float32)
    bo = np.asarray(bo, np.float32)

    nc = _get_nc()
    sc = 1.0 / np.sqrt(E // H)
    # [m, p, b, d]: lhsT tile for Q M-tile m, e-block b
    wq_h = np.ascontiguousarray(
        (Wq * sc).reshape(EB, P, H, P).transpose(2, 1, 0, 3)).astype(bft)
    # K / V column split of Wkv ([K0 V0 K1 V1 ...] blocks of 128)
    kcols = np.concatenate(
        [np.arange(g * 2 * P, g * 2 * P + P) for g in range(G)])
    vcols = kcols + P
    wk_h = np.ascontiguousarray(
        Wkv[:, kcols].reshape(EB, P, G * P).transpose(1, 0, 2)).astype(bft)
    wv_h = np.ascontiguousarray(
        Wkv[:, vcols].reshape(EB, P, G * P).transpose(1, 0, 2)).astype(bft)
    wo_h = np.ascontiguousarray(
        Wo.reshape(EB, P, E).transpose(1, 0, 2)).astype(bft)
    bq_h = np.ascontiguousarray((bq * sc).reshape(H, P).T)
    bk_h = np.ascontiguousarray(bkv[kcols].reshape(G, P).T)
    bv_h = np.ascontiguousarray(bkv[vcols].reshape(1, G * P))
    bo_h = np.ascontiguousarray(bo.reshape(1, E))
    oc4_h = np.ascontiguousarray(
        np.broadcast_to(np.eye(4, dtype=np.float32), (P, 4, 4))).astype(bft)
    selr_h = np.ascontiguousarray(
        np.repeat(np.eye(4, dtype=np.float32), P, axis=1).reshape(4, 4, P))

    in_maps = []
    for c in range(NCORES):
        b, q = divmod(c, 4)
        xT = x[b].T.astype(bft)  # [e, s]
        order = [q] + [i for i in range(4) if i != q]
        # [p, chunk, b, 512] with this core's s_q quarter as chunk 0
        xt_h = np.ascontiguousarray(
            np.stack([xT[:, 512 * i:512 * (i + 1)].reshape(EB, P, 512)
                      for i in order], axis=0).transpose(2, 0, 1, 3))
        in_maps.append({"x": xt_h, "wq": wq_h, "wk": wk_h, "wv": wv_h,
                        "wo": wo_h, "bq": bq_h, "bk": bk_h, "bvr": bv_h,
                        "bo": bo_h, "oc4d": oc4_h, "selrd": selr_h})

    res = run_bass_kernel_spmd(nc, in_maps, core_ids=list(range(NCORES)),
                               trace=TRACE)
    LAST_RESULT = res

    outf = np.empty((2, S, E), np.float32)
    for c in range(NCORES):
        b, q = divmod(c, 4)
        outf[b, 512 * q:512 * (q + 1), :] = res.results[c]["out"]
    return outf


# revision 29
# speedup vs baseline: 1.7201x; 1.0863x over previous
"""Grouped Query Attention on 8 TRN2 NeuronCores.

Sharding: batch x s_q-quarter (core c -> batch c//4, query rows
[512*(c%4), 512*(c%4+1))). Each core computes the Q projection for its
512 query rows, the full KV projection for its batch (duplicated across
the 4 cores of that batch -- cheaper than collectives), attention for
all 16 heads over its query rows, and the output projection for a
disjoint [512, 2048] slice of the output. Unsharding is concatenation;
no collectives.

v2 (from trace analysis of the f32 baseline, 724us):
- bf16 inputs/weights (host-cast): halves HBM traffic, same PE rate.
- All intermediates (Q^T, K^T, V, attn out) stay SBUF-resident; the
  f32 baseline spilled Q^T/KV^T to DRAM and re-streamed them.
- V is produced directly in [s, d] layout by swapping matmul operand
  roles (lhsT = x^T tile, rhs = Wv block), eliminating 64 PE
  transposes.
- Scores for two s_k tiles share one [128, 1024] PSUM tile, so exp
  runs as 8 wide ACT ops per head instead of 16 (ACT was near the
  per-head PE time).
- Softmax denominators: e-tiles are tree-summed on DVE + Pool (idle
  engines) and reduced with ONE f32r ones-matmul per head, replacing
  16 accumulating [1,512] PE matmuls per head (~55us of PE time).
- 1/sqrt(128) folded into Wq/bq on host; normalization applied by
  broadcasting 1/l via a K=1 f32r matmul.
"""

import numpy as np

E = 2048
S = 2048
P = 128
H = 16
G = 4
SQ = 512          # query rows per core
EB = E // P       # 16 e-blocks (contraction tiles)
NCORES = 8

_NC = None
TRACE = False
LAST_RESULT = None


def _build():
    import concourse.bacc as bacc
    import concourse.mybir as mybir
    import concourse.tile as tile

    f32 = mybir.dt.float32
    f32r = mybir.dt.float32r
    bf16 = mybir.dt.bfloat16
    EXP = mybir.ActivationFunctionType.Exp

    nc = bacc.Bacc("TRN2", target_bir_lowering=False, debug=False,
                   num_devices=NCORES)

    x = nc.declare_dram_parameter("x", [P, EB, 512], bf16, isOutput=False).ap()
    wq = nc.declare_dram_parameter("wq", [H, P, EB, P], bf16, isOutput=False).ap()
    wk = nc.declare_dram_parameter("wk", [P, EB, G * P], bf16, isOutput=False).ap()
    wv = nc.declare_dram_parameter("wv", [P, EB, G * P], bf16, isOutput=False).ap()
    wo = nc.declare_dram_parameter("wo", [P, EB, E], bf16, isOutput=False).ap()
    bq = nc.declare_dram_parameter("bq", [P, H], f32, isOutput=False).ap()
    bk = nc.declare_dram_parameter("bk", [P, G], f32, isOutput=False).ap()
    bvr = nc.declare_dram_parameter("bvr", [1, G * P], f32, isOutput=False).ap()
    bo = nc.declare_dram_parameter("bo", [1, E], f32, isOutput=False).ap()
    oc4d = nc.declare_dram_parameter("oc4d", [P, 4, 4], bf16, isOutput=False).ap()
    selrd = nc.declare_dram_parameter("selrd", [4, 4, P], f32, isOutput=False).ap()
    out = nc.declare_dram_parameter("out", [SQ, E], f32, isOutput=True).ap()

    def r(ap):
        return ap.bitcast(f32r)

    with tile.TileContext(nc) as tc, \
         nc.allow_low_precision(reason="bf16 intermediates; end-to-end rel-err checked"):
        with tc.tile_pool(name="consts", bufs=1) as cp, \
             tc.tile_pool(name="keep", bufs=1) as kp:
            onec = cp.tile([P, 1], bf16, tag="onec")
            nc.vector.memset(onec, 1.0)
            oner0 = cp.tile([1, P], f32, tag="oner0")
            nc.vector.memset(oner0, 1.0)
            oner = cp.tile([1, P], f32, tag="oner")
            nc.vector.tensor_copy(r(oner), oner0)
            # indicator constants for batched softmax-denominator rows
            # (host-prepared): oc4[:, hl] is [128,4] with column hl
            # all-ones (ones-matmul lhsT -> row hl of a [4,512] psum
            # tile); selr[:, hl] is [4,128] with row hl all-ones
            # (broadcast-back lhsT).
            oc4 = cp.tile([P, 4, 4], bf16, tag="oc4")
            nc.sync.dma_start(oc4, oc4d)
            selr = cp.tile([4, 4, P], f32, tag="selr")
            nc.sync.dma_start(r(selr), r(selrd))
            bq_s = cp.tile([P, H], f32, tag="bqs")
            nc.sync.dma_start(bq_s, bq)
            bk_s = cp.tile([P, G], f32, tag="bks")
            nc.sync.dma_start(bk_s, bk)
            bv_s = cp.tile([1, G * P], f32, tag="bvs")
            nc.sync.dma_start(r(bv_s), r(bvr))
            bo_s = cp.tile([1, E], f32, tag="bos")
            nc.sync.dma_start(r(bo_s), r(bo))

            qT = kp.tile([P, H, SQ], bf16, tag="qt")    # Q^T per head block
            kT = kp.tile([P, G, S], bf16, tag="kt")     # K^T per group
            Vs = kp.tile([P, EB, G * P], bf16, tag="vs")  # V[s_tile, 4 groups*128]
            OT = kp.tile([P, H, SQ], bf16, tag="ot")    # normalized attn out

            # ---- Phase 1: projections. Each core holds ONLY its own
            # s-quarter of x^T. It computes K^T/V for that quarter, then
            # an AllGather across the 4 cores of its batch assembles the
            # full-sequence K^T/V while the PE crunches the Q projection
            # (the gather concatenates rank-major = natural s order).
            with tc.tile_pool(name="xp", bufs=1) as xp, \
                 tc.tile_pool(name="wqp", bufs=3) as wqp, \
                 tc.tile_pool(name="wkvp", bufs=1) as wkvp, \
                 tc.tile_pool(name="kvq", bufs=1) as kvq, \
                 tc.tile_pool(name="dramp", bufs=1, space="DRAM") as dramp, \
                 tc.tile_pool(name="ps1", bufs=3, space="PSUM") as ps1, \
                 tc.tile_pool(name="ps1v", bufs=3, space="PSUM") as ps1v:
                x0 = xp.tile([P, EB, 512], bf16, tag="x0")
                nc.sync.dma_start(x0, x)
                wk_s = wkvp.tile([P, EB, G * P], bf16, tag="wks")
                nc.sync.dma_start(wk_s, wk)
                wv_s = wkvp.tile([P, EB, G * P], bf16, tag="wvs")
                nc.sync.dma_start(wv_s, wv)

                kTq = kvq.tile([P, G, 512], bf16, tag="ktq")
                Vq = kvq.tile([P, 4, G * P], bf16, tag="vq")
                kv_in = dramp.tile([2, P, G, 512], bf16, tag="kvin")
                kv_ag = dramp.tile([4, 2, P, G, 512], bf16, tag="kvag")

                # K^T / V for this core's own s-quarter
                for g in range(G):
                    ps = ps1.tile([P, 512], f32, tag="ps")
                    for b in range(EB):
                        nc.tensor.matmul(
                            ps, wk_s[:, b, g * P:(g + 1) * P], x0[:, b],
                            start=(b == 0), stop=(b == EB - 1))
                    nc.vector.tensor_scalar_add(kTq[:, g], ps,
                                                bk_s[:, g:g + 1])
                for t in range(4):
                    ps = ps1v.tile([P, G * P], f32, tag="psv")
                    nc.tensor.matmul(ps, r(oner), r(bv_s),
                                     start=True, stop=False)
                    for b in range(EB):
                        nc.tensor.matmul(
                            ps, x0[:, b, t * P:(t + 1) * P], wv_s[:, b],
                            start=False, stop=(b == EB - 1))
                    nc.scalar.copy(Vq[:, t], ps)

                nc.sync.dma_start(kv_in[0], kTq)
                nc.sync.dma_start(kv_in[1], Vq)
                nc.gpsimd.collective_compute(
                    "AllGather",
                    mybir.AluOpType.bypass,
                    replica_groups=[[0, 1, 2, 3], [4, 5, 6, 7]],
                    ins=[kv_in.opt()],
                    outs=[kv_ag.opt()],
                )

                # Q projection: QT[d, s_q] for 16 head blocks (overlaps
                # with the AllGather)
                wqts = [wqp.tile([P, EB, P], bf16, tag="wqm", name="wqm")
                        for m in range(3)]
                for m in range(3):
                    nc.sync.dma_start(wqts[m], wq[m])
                for m in range(H):
                    wqm = wqts[m]
                    if m + 3 < H:
                        w_next = wqp.tile([P, EB, P], bf16, tag="wqm")
                        nc.sync.dma_start(w_next, wq[m + 3])
                        wqts.append(w_next)
                    ps = ps1.tile([P, SQ], f32, tag="ps")
                    for b in range(EB):
                        nc.tensor.matmul(ps, wqm[:, b], x0[:, b],
                                         start=(b == 0), stop=(b == EB - 1))
                    nc.vector.tensor_scalar_add(qT[:, m], ps, bq_s[:, m:m + 1])

                # unpack the gathered K^T / V (natural s order)
                for q in range(4):
                    nc.sync.dma_start(kT[:, :, 512 * q:512 * (q + 1)],
                                      kv_ag[q, 0])
                    nc.sync.dma_start(Vs[:, 4 * q:4 * (q + 1), :],
                                      kv_ag[q, 1])

            # ---- Phase 2: attention. scores^T for two s_k tiles land in
            # one [128,1024] PSUM tile -> one exp -> two attn@V matmuls.
            # Denominator: DVE+Pool tree-sum of e tiles, one f32r
            # ones-matmul, reciprocal, K=1 broadcast matmul.
            with tc.tile_pool(name="wop", bufs=1) as wop:
                wo_s = wop.tile([P, EB, E], bf16, tag="wos")
                nc.sync.dma_start(wo_s, wo)  # prefetch for phase 3

                with tc.tile_pool(name="exq", bufs=5) as exq, \
                     tc.tile_pool(name="accp", bufs=8) as accp, \
                     tc.tile_pool(name="lsb", bufs=2) as lsb, \
                     tc.tile_pool(name="psfp", bufs=5) as psfp, \
                     tc.tile_pool(name="pscp", bufs=2, space="PSUM") as pscp, \
                     tc.tile_pool(name="psop", bufs=2, space="PSUM") as psop, \
                     tc.tile_pool(name="pslp", bufs=1, space="PSUM") as pslp, \
                     tc.tile_pool(name="psbp", bufs=1, space="PSUM") as psbp:
                    # pipelined per-head state
                    pend = [None]   # (hl, a01, a23, psl4) awaiting psl matmuls
                    recq = []       # [(psl4, li4)] groups awaiting reciprocal
                    finq = []       # [(h, hl, psof, li4)] awaiting plb+mul

                    def emit_psl():
                        # denominator rows: 4 accumulating [4,512] matmuls
                        # straight off the two level-2 tree partials
                        hl_p, a01_p, a23_p, psl4_p = pend[0]
                        for k, rhs in enumerate(
                                (a01_p[:, 0:512], a01_p[:, 512:1024],
                                 a23_p[:, 0:512], a23_p[:, 512:1024])):
                            nc.tensor.matmul(psl4_p, oc4[:, hl_p], rhs,
                                             start=(hl_p == 0 and k == 0),
                                             stop=(hl_p == 3 and k == 3))
                        pend[0] = None

                    def emit_recip():
                        psl4_p, li4, staged = recq.pop(0)
                        nc.vector.reciprocal(r(li4), psl4_p)
                        finq.extend(staged)

                    def emit_fin():
                        h_p, hl_p, psof, li4 = finq.pop(0)
                        plb = psbp.tile([P, SQ], f32, tag="plb")
                        nc.tensor.matmul(plb, r(selr[:, hl_p]), r(li4),
                                         start=True, stop=True)
                        lbs = lsb.tile([P, SQ], f32, tag="lbs")
                        nc.vector.tensor_copy(lbs, plb)
                        nc.gpsimd.tensor_mul(OT[:, h_p], psof, lbs)

                    for g in range(G):
                        psl4 = pslp.tile([4, SQ], f32, tag="psl4")
                        li4 = lsb.tile([4, SQ], f32, tag="li4")
                        grp_stage = []
                        for hl in range(4):
                            h = 4 * g + hl
                            qh = qT[:, h]
                            pso = psop.tile([P, SQ], f32, tag="pso")
                            exps = [None] * 8

                            def sc(i, g=g, qh=qh, exps=exps):
                                # two s_k tiles -> one [128,1024] psum
                                p = pscp.tile([P, 1024], f32, tag="psc")
                                for u in range(2):
                                    t = 2 * i + u
                                    nc.tensor.matmul(
                                        p[:, 512 * u:512 * (u + 1)],
                                        kT[:, g, t * P:(t + 1) * P], qh,
                                        start=True, stop=True)
                                e = exq.tile([P, 1024], bf16, tag="ex")
                                nc.scalar.activation(e, p, EXP)
                                exps[i] = e

                            def av(i, g=g, pso=pso, exps=exps):
                                e = exps[i]
                                for u in range(2):
                                    t = 2 * i + u
                                    nc.tensor.matmul(
                                        pso, Vs[:, t, g * P:(g + 1) * P],
                                        e[:, 512 * u:512 * (u + 1)],
                                        start=(i == 0 and u == 0),
                                        stop=(i == 7 and u == 1))

                            sc(0)
                            sc(1)
                            lvl1 = []
                            for i in range(8):
                                if i + 2 < 8:
                                    sc(i + 2)
                                if i == 1 and pend[0] is not None:
                                    emit_psl()
                                if i == 3 and recq:
                                    emit_recip()
                                if i in (5, 7) and finq:
                                    emit_fin()
                                av(i)
                                if i % 2 == 1:
                                    a = accp.tile([P, 1024], bf16, tag="acc")
                                    # early tiles on the slow engine, late
                                    # tiles on DVE (short tail chain)
                                    eng = nc.gpsimd if i in (1, 3) \
                                        else nc.vector
                                    eng.tensor_add(a, exps[i - 1], exps[i])
                                    lvl1.append(a)
                            a01 = accp.tile([P, 1024], bf16, tag="acc")
                            nc.gpsimd.tensor_add(a01, lvl1[0], lvl1[1])
                            a23 = accp.tile([P, 1024], bf16, tag="acc")
                            nc.vector.tensor_add(a23, lvl1[2], lvl1[3])
                            # spill pso to SBUF (frees PSUM for next head)
                            psof = psfp.tile([P, SQ], f32, tag="psof")
                            nc.vector.tensor_copy(psof, pso)
                            pend[0] = (hl, a01, a23, psl4)
                            grp_stage.append((h, hl, psof, li4))
                        recq.append((psl4, li4, grp_stage))
                    emit_psl()
                    while recq:
                        emit_recip()
                    while finq:
                        emit_fin()

                # ---- Phase 3: output projection. out[s_q, eo] accumulates
                # over 16 head blocks; bias seeded via a K=1 ones matmul.
                with tc.tile_pool(name="obp", bufs=3) as obp, \
                     tc.tile_pool(name="ps3", bufs=2, space="PSUM") as ps3p:
                    for n in range(4):
                        for ms in range(4):
                            ps = ps3p.tile([P, 512], f32, tag="ps")
                            nc.tensor.matmul(
                                ps, r(oner), r(bo_s[:, 512 * n:512 * (n + 1)]),
                                start=True, stop=False)
                            for k in range(EB):
                                nc.tensor.matmul(
                                    ps, OT[:, k, ms * P:(ms + 1) * P],
                                    wo_s[:, k, 512 * n:512 * (n + 1)],
                                    start=False, stop=(k == EB - 1))
                            ob = obp.tile([P, 512], f32, tag="ob")
                            nc.vector.tensor_copy(ob, ps)
                            nc.sync.dma_start(
                                out[ms * P:(ms + 1) * P, 512 * n:512 * (n + 1)], ob)

    nc.compile()
    return nc


def _get_nc():
    global _NC
    if _NC is None:
        _NC = _build()
    return _NC


def kernel(x, Wq, bq, Wkv, bkv, Wo, bo):
    from concourse.bass_utils import run_bass_kernel_spmd
    import ml_dtypes
    global LAST_RESULT

    bft = ml_dtypes.bfloat16
    x = np.asarray(x, np.float32)
    Wq = np.asarray(Wq, np.float32)
    bq = np.asarray(bq, np.float32)
    Wkv = np.asarray(Wkv, np.float32)
    bkv = np.asarray(bkv, np.float32)
    Wo = np.asarray(Wo, np.float32)
    bo = np.asarray(bo, np.float32)

    nc = _get_nc()
    sc = 1.0 / np.sqrt(E // H)
    # [m, p, b, d]: lhsT tile for Q M-tile m, e-block b
    wq_h = np.ascontiguousarray(
        (Wq * sc).reshape(EB, P, H, P).transpose(2, 1, 0, 3)).astype(bft)
    # K / V column split of Wkv ([K0 V0 K1 V1 ...] blocks of 128)
    kcols = np.concatenate(
        [np.arange(g * 2 * P, g * 2 * P + P) for g in range(G)])
    vcols = kcols + P
    wk_h = np.ascontiguousarray(
        Wkv[:, kcols].reshape(EB, P, G * P).transpose(1, 0, 2)).astype(bft)
    wv_h = np.ascontiguousarray(
        Wkv[:, vcols].reshape(EB, P, G * P).transpose(1, 0, 2)).astype(bft)
    wo_h = np.ascontiguousarray(
        Wo.reshape(EB, P, E).transpose(1, 0, 2)).astype(bft)
    bq_h = np.ascontiguousarray((bq * sc).reshape(H, P).T)
    bk_h = np.ascontiguousarray(bkv[kcols].reshape(G, P).T)
    bv_h = np.ascontiguousarray(bkv[vcols].reshape(1, G * P))
    bo_h = np.ascontiguousarray(bo.reshape(1, E))
    oc4_h = np.ascontiguousarray(
        np.broadcast_to(np.eye(4, dtype=np.float32), (P, 4, 4))).astype(bft)
    selr_h = np.ascontiguousarray(
        np.repeat(np.eye(4, dtype=np.float32), P, axis=1).reshape(4, 4, P))

    in_maps = []
    for c in range(NCORES):
        b, q = divmod(c, 4)
        # this core's own s-quarter of x^T, [p, eb, 512]
        xt_h = np.ascontiguousarray(
            x[b].T[:, 512 * q:512 * (q + 1)].astype(bft)
            .reshape(EB, P, 512).transpose(1, 0, 2))
        in_maps.append({"x": xt_h, "wq": wq_h, "wk": wk_h, "wv": wv_h,
                        "wo": wo_h, "bq": bq_h, "bk": bk_h, "bvr": bv_h,
                        "bo": bo_h, "oc4d": oc4_h, "selrd": selr_h})

    res = run_bass_kernel_spmd(nc, in_maps, core_ids=list(range(NCORES)),
                               trace=TRACE)
    LAST_RESULT = res

    outf = np.empty((2, S, E), np.float32)
    for c in range(NCORES):
        b, q = divmod(c, 4)
        outf[b, 512 * q:512 * (q + 1), :] = res.results[c]["out"]
    return outf
